# revision 23
# baseline (speedup 1.0000x reference)
"""Trainium2 Bass kernel for nn_Att_61881888801149 (sparse_attention).

Math (per batch b):
    q = x @ Wq + bq                  [L, Cr]
    k = x @ Wk + bk                  [L, Cr]
    v = x @ Wv + bv                  [L, C]
    pos = (rel_h + rel_w).reshape(Cr, L)
    S = q @ (k^T + pos)              [L, L]   (queries l, keys m)
    attn = softmax(S, axis=0)        (normalized over the QUERY axis l)
    out = attn @ v                   [L, C]

Because the softmax axis (l) is orthogonal to the bmm contraction axis (m):
    out[l, c] = sum_m  E[l, m] * v[m, c] / colsum[m]
with E = exp(S) (no max subtraction needed - scores are small), and
colsum[m] = sum_l E[l, m].

Sharding: 8 cores = 4 batches x 2 key-halves (m in [0,2048) or [2048,4096)).
Host sums the two partial outputs per batch.  SPMD trick: the host rotates
xT's columns per-core so each core's m-half is always columns 0:2048; the
output columns (l, also rotated) are un-rotated on the host.

On-core layout: everything is computed transposed:
    qT  [Cr, L]  = Wq^T @ xT + bq
    kpT [Cr, M]  = Wk^T @ xTm + (pos + bk)         (pos+bk folded on host)
    vb  [M, C]   = xTm^T @ Wv + bv (rank-1 ones matmul for the bias)
    ST  [M, L]   = kpT^T @ qT      -> exp (ACT, fused colsum accumulation)
    E   [M, L]   bf16, resident in SBUF (16MB)
    vbw [M, C]   = vb * (1/colsum) per row, bf16
    outT[C, L]   = vbw^T @ E       (PSUM accumulation over m-blocks)
"""

import sys

for _p in ("/opt/trn_rl_repo", "/root/.axon_site/_ro/trn_rl_repo"):
    if _p not in sys.path:
        sys.path.append(_p)

import numpy as np

B, L, C, Cr = 4, 4096, 256, 32
MH = L // 2  # per-core key-half size (2048)
NCORES = 8

_CACHE = {}


def build_nc(L=L, C=C, Cr=Cr, M=MH):
    import concourse.bass as bass
    import concourse.tile as tile
    from concourse import mybir
    from concourse.tile_rust import add_dep_helper

    FP32 = mybir.dt.float32
    FP32R = mybir.dt.float32r
    FP16 = mybir.dt.float16
    Exp = mybir.ActivationFunctionType.Exp
    # E is stored as fp16 exp(S - OFF).  The offset cancels exactly in
    # out = E' @ (v / colsum(E')) and keeps exp(S) inside fp16 range:
    # real-data S in [-19, 19.44], colmax in [3.6, 19.44] -> E' <= e^7.9,
    # vbw' <= |v| * e^{OFF - colmax_min} ~ 4e3, both with >= 16x margin.
    EXP_OFF = 11.5

    assert C == 256 and Cr == 32
    assert L % 1024 == 0 and M % 512 == 0
    NMB = M // 128          # m-blocks per core
    NLG = L // 512          # l-groups for phase 2
    SG = 1024               # phase-1 ACT exp chunk width
    NSG = L // SG           # stats groups per m-block

    # xin columns: xT rows 0:128 at 0:L | xT rows 128:256 at L:2L | then
    # wq0 +0:32 | wq1 +32:64 | wk0 +64:96 | wk1 +96:128 | wv0 +128:384 |
    # wv1 +384:640 | bq(row0) +640:672 | bv(row0) +672:928 | pos +928:+928+M |
    # ones(row0) +928+M:+1440+M
    FPW = 928 + M + 512
    XW = 2 * L + FPW

    nc = bass.Bass()
    xin_d = nc.dram_tensor("xin", [128, XW], FP32R, kind="ExternalInput")
    outT_d = nc.dram_tensor("outT", [C, L], FP32, kind="ExternalOutput")

    with tile.TileContext(nc) as tc:
        with (
            tc.tile_pool(name="persist", bufs=1) as persist,
            tc.tile_pool(name="psum", bufs=1, space="PSUM") as psum,
        ):
            qT = persist.tile([Cr, L], FP16)
            kpT = persist.tile([Cr, M], FP16)
            vb = persist.tile([128, NMB, C], FP32)
            vbw = persist.tile([128, NMB, C], FP16)
            stats = persist.tile([128, NMB, NSG], FP32)
            colsum = persist.tile([128, NMB], FP32)
            wrec = persist.tile([128, NMB], FP32)
            expoff = persist.tile([128, 1], FP32)
            nc.vector.memset(expoff[:], -EXP_OFF)
            # tiny fp16 weights tile for Ldweights "carrier" instructions:
            # a PE-queue op that absorbs the same-engine WAW semaphore wait
            # of a PSUM slot being re-opened, so the slot-opening Matmult
            # (1-sem-wait ISA budget) only carries the cross-engine WAR.
            wdum = persist.tile([1, 1], FP16)
            nc.vector.memset(wdum[:], 0.0)
            pscr = persist.tile([1, 2], FP32)

            def carrier(dep):
                if dep is None:
                    return None
                c = nc.tensor.ldweights(wdum[:])
                add_dep_helper(c.ins, dep.ins, sync=True,
                               reason="psum slot WAW carrier")
                return c

            def anchor(mm, c):
                # give the carrier a descendant so the scheduler ticks it
                # (and keeps it ordered before the matmul group)
                if c is not None:
                    add_dep_helper(mm.ins, c.ins, sync=False,
                                   reason="carrier anchor")
                return mm

            # ---------------- prologue ----------------
            with tc.tile_pool(name="prolog", bufs=1) as pp:
                xin = pp.tile([128, XW], FP32R)
                xin_dma = nc.sync.dma_start(xin[:], xin_d[:])
                # engine warm-ups: let DVE/ACT observe the input-DMA queue
                # semaphore on a cheap op early.
                dvew = pp.tile([1, 2], FP32)
                nc.vector.tensor_copy(dvew[0:1, 0:1], xin[0:1, 0:1].bitcast(FP32))

                xt = xin[:, 0:2 * L].rearrange("p (two l) -> p two l", two=2)
                wp = xin[:, 2 * L:]
                wq0, wq1 = wp[:, 0:32], wp[:, 32:64]
                wk0, wk1 = wp[:, 64:96], wp[:, 96:128]
                wv0, wv1 = wp[:, 128:384], wp[:, 384:640]
                bq = wp[0:1, 640:672]
                bv = wp[0:1, 672:928]
                pos = wp[0:32, 928:928 + M].bitcast(FP32)
                ones = wp[0:1, 928 + M:1440 + M]

                hist_q = [None, None]
                hist_v = [None, None]

                # qT = Wq^T @ xT + bq  (bias via rank-1 ones matmul)
                for j in range(L // 512):
                    sl = slice(j * 512, (j + 1) * 512)
                    cr_ = carrier(hist_q[j % 2])
                    psq = psum.tile([Cr, 512], FP32, tag="psq", bufs=2)
                    anchor(nc.tensor.matmul(psq[:], wq0, xt[:, 0, sl],
                                            start=True, stop=False), cr_)
                    nc.tensor.matmul(psq[:], wq1, xt[:, 1, sl],
                                     start=False, stop=False)
                    nc.tensor.matmul(psq[:], bq, ones[:],
                                     start=False, stop=True)
                    hist_q[j % 2] = nc.scalar.copy(qT[:, sl], psq[:])

                # kpT = Wk^T @ xTm + pos  (pos already includes bk)
                for j in range(M // 512):
                    sl = slice(j * 512, (j + 1) * 512)
                    cr_ = carrier(hist_q[j % 2])
                    psk = psum.tile([Cr, 512], FP32, tag="psq", bufs=2)
                    anchor(nc.tensor.matmul(psk[:], wk0, xt[:, 0, sl],
                                            start=True, stop=False), cr_)
                    nc.tensor.matmul(psk[:], wk1, xt[:, 1, sl],
                                     start=False, stop=True)
                    hist_q[j % 2] = nc.vector.tensor_add(kpT[:, sl], psk[:],
                                                         pos[:, sl])

                # vb = xTm^T @ Wv + bv  (bias via rank-1 ones matmul)
                for mb in range(NMB):
                    msl = slice(mb * 128, (mb + 1) * 128)
                    cr_ = carrier(hist_v[mb % 2])
                    psv = psum.tile([128, C], FP32, tag="psv", bufs=2)
                    anchor(nc.tensor.matmul(psv[:], xt[:, 0, msl], wv0,
                                            start=True, stop=False), cr_)
                    nc.tensor.matmul(psv[:], xt[:, 1, msl], wv1,
                                     start=False, stop=False)
                    vmm = nc.tensor.matmul(psv[:], ones[0:1, 0:128],
                                           bv, start=False, stop=True)
                    last_vb = nc.vector.tensor_copy(vb[:, mb, :], psv[:])
                    hist_v[mb % 2] = last_vb

            # ---------------- phase 1 + 2 ----------------
            with tc.tile_pool(name="epool", bufs=1) as epool:
                E = epool.tile([128, NMB, L], FP16)
                hist_st = [None, None]

                first_ps = psum.tile([128, SG], FP32, tag="st", bufs=2)
                # phase-entry absorbers: make PE observe the last DVE (vb)
                # and ACT (qT) ticks, one semaphore each (banks A and B of
                # first_ps, so they do not WAW-chain on each other).
                b1 = nc.tensor.matmul(first_ps[0:1, 0:256],
                                      vb[:, NMB - 1, 0:1], vb[:, NMB - 1, 0:256],
                                      start=True, stop=True)
                b2 = nc.tensor.matmul(first_ps[0:1, 512:1024],
                                      qT[:, 0:1], qT[:, L - 512:L],
                                      start=True, stop=True)
                add_dep_helper(b2.ins, b1.ins, sync=False,
                               reason="schedule b2 as the last prologue PE op")
                # ACT absorbers for the SBUF pool boundary (prolog zone
                # -> E zone): split the {DVE, PE, ACT-self} boundary waits
                # over three single-wait copies so the first exp
                # (Activation: 2-sem-wait budget) stays within budget.
                p0 = nc.scalar.copy(pscr[0:1, 0:1], expoff[0:1, 0:1])
                add_dep_helper(p0.ins, xin_dma.ins, sync=True,
                               reason="ACT observes input-DMA queue")
                p1 = nc.scalar.copy(pscr[0:1, 0:1], expoff[0:1, 0:1])
                add_dep_helper(p1.ins, last_vb.ins, sync=True,
                               reason="ACT observes DVE tail")
                add_dep_helper(p1.ins, p0.ins, sync=False, reason="chain")
                p2 = nc.scalar.copy(pscr[0:1, 1:2], expoff[0:1, 0:1])
                add_dep_helper(p2.ins, b2.ins, sync=True,
                               reason="ACT observes PE tail")
                add_dep_helper(p2.ins, p1.ins, sync=False, reason="chain")

                for mb in range(NMB):
                    kp_sl = kpT[:, mb * 128:(mb + 1) * 128]
                    for g in range(NSG):
                        idx = mb * NSG + g
                        if idx == 0:
                            ps, cr_ = first_ps, None
                        else:
                            cr_ = carrier(hist_st[idx % 2])
                            ps = psum.tile([128, SG], FP32, tag="st", bufs=2)
                        last = None
                        for j in range(SG // 512):
                            lsl = slice(g * SG + j * 512, g * SG + (j + 1) * 512)
                            last = nc.tensor.matmul(
                                ps[:, j * 512:(j + 1) * 512],
                                kp_sl, qT[:, lsl], start=True, stop=True)
                            if j == 0:
                                anchor(last, cr_)
                        last_exp = nc.scalar.activation(
                            E[:, mb, g * SG:(g + 1) * SG], ps[:], Exp,
                            bias=expoff[:],
                            accum_out=stats[:, mb, g:g + 1])
                        hist_st[idx % 2] = last_exp
                    nc.vector.reduce_sum(colsum[:, mb:mb + 1],
                                         stats[:, mb, :],
                                         axis=mybir.AxisListType.X)
                    nc.vector.reciprocal(wrec[:, mb:mb + 1],
                                         colsum[:, mb:mb + 1])
                    last_vbw = nc.vector.tensor_scalar_mul(
                        vbw[:, mb, :], vb[:, mb, :], wrec[:, mb:mb + 1])

                # phase-2 entry absorbers (ACT first: matches the st-slot
                # WAR semaphore; banks B then A to avoid a WAW chain).
                nidx = NMB * NSG
                cr_ = carrier(last_exp)
                ps_a = psum.tile([128, SG], FP32, tag="st", bufs=2)
                a2 = nc.tensor.matmul(ps_a[0:1, 512:1024],
                                      E[:, NMB - 1, 0:1],
                                      E[:, NMB - 1, L - 512:L],
                                      start=True, stop=True)
                anchor(a2, cr_)
                a1 = nc.tensor.matmul(ps_a[0:1, 0:256],
                                      vbw[:, NMB - 1, 0:1],
                                      vbw[:, NMB - 1, 0:256],
                                      start=True, stop=True)
                hist_st[nidx % 2] = None
                nidx += 1

                with tc.tile_pool(name="stage", bufs=3) as stage:
                    for lg in range(NLG):
                        lsl = slice(lg * 512, (lg + 1) * 512)
                        for ch in range(C // 128):
                            cr_ = carrier(hist_st[nidx % 2])
                            po = psum.tile([128, SG], FP32, tag="st", bufs=2)
                            last = None
                            for mb in range(NMB):
                                last = nc.tensor.matmul(
                                    po[:, 0:512],
                                    vbw[:, mb, ch * 128:(ch + 1) * 128],
                                    E[:, mb, lsl],
                                    start=(mb == 0), stop=(mb == NMB - 1))
                                if mb == 0:
                                    anchor(last, cr_)
                            nidx += 1
                            so = stage.tile([128, 512], FP32, tag="so")
                            nc.scalar.copy(so[0:1, 0:1], expoff[0:1, 0:1])
                            hist_st[(nidx - 1) % 2] = nc.scalar.copy(
                                so[:], po[:, 0:512])
                            nc.sync.dma_start(
                                outT_d[ch * 128:(ch + 1) * 128, lsl], so[:])

    return nc


def _fixup_waits(nc):
    """Walrus codegen on this toolchain allows only ~1 semaphore wait per
    TPB instruction (2 for DMACopy).  Hoist excess waits into standalone
    single-wait EventSemaphore instructions inserted just before the
    over-budget instruction on the same engine (same-stream ordering makes
    this semantics-preserving)."""
    from concourse import mybir

    budget_by_type = {}
    n = 0
    for fn in nc.m.functions:
        for blk in fn.blocks:
            insts = blk.instructions
            i = 0
            while i < len(insts):
                inst = insts[i]
                si = getattr(inst, "sync_info", None)
                if si is None:
                    i += 1
                    continue
                waits = list(si.on_wait)
                budget = budget_by_type.get(type(inst).__name__, 1)
                if len(waits) <= budget:
                    i += 1
                    continue
                extra, keep = waits[:-budget], waits[-budget:]
                for w in extra:
                    es = mybir.InstEventSemaphore(
                        name=f"es_waitfix_{n}", ins=[], outs=[])
                    n += 1
                    es.engine = inst.engine
                    es.sync_info = mybir.SyncInfo(on_wait=[w], on_update=[])
                    insts.insert(i, es)
                    i += 1
                inst.sync_info = mybir.SyncInfo(
                    on_wait=keep, on_update=list(si.on_update))
                i += 1


def _build_and_fix(**kw):
    nc = build_nc(**kw)
    _fixup_waits(nc)
    return nc


def _get_nc(key, **kw):
    if key not in _CACHE:
        _CACHE[key] = _build_and_fix(**kw)
    return _CACHE[key]


def _prep_core_inputs(x, rel_h, rel_w, Wq, bq, Wk, bk, Wv, bv):
    """Build the 8 per-core input maps (host-side sharding / layout prep)."""
    x = np.asarray(x, dtype=np.float32)
    pos = (np.asarray(rel_h, np.float32) + np.asarray(rel_w, np.float32))
    pos = pos.reshape(Cr, L) + np.asarray(bk, np.float32).reshape(Cr, 1)
    FPW = 928 + MH + 512
    wpacks = []
    for h in range(2):
        wp = np.zeros((128, FPW), np.float32)
        wp[:, 0:32] = np.asarray(Wq, np.float32)[0:128]
        wp[:, 32:64] = np.asarray(Wq, np.float32)[128:256]
        wp[:, 64:96] = np.asarray(Wk, np.float32)[0:128]
        wp[:, 96:128] = np.asarray(Wk, np.float32)[128:256]
        wp[:, 128:384] = np.asarray(Wv, np.float32)[0:128]
        wp[:, 384:640] = np.asarray(Wv, np.float32)[128:256]
        wp[0, 640:672] = np.asarray(bq, np.float32).ravel()
        wp[0, 672:928] = np.asarray(bv, np.float32).ravel()
        wp[0:32, 928:928 + MH] = pos[:, h * MH:(h + 1) * MH]
        wp[0, 928 + MH:1440 + MH] = 1.0
        wpacks.append(wp)

    in_maps = []
    for i in range(NCORES):
        b, h = divmod(i, 2)
        xT = x[b].T  # [C, L]
        if h == 1:
            xT = np.concatenate([xT[:, MH:], xT[:, :MH]], axis=1)
        xin = np.ascontiguousarray(
            np.concatenate([xT[0:128], xT[128:256], wpacks[h]], axis=1))
        in_maps.append({"xin": xin})
    return in_maps


def _combine(results):
    """results: list of 8 out_maps -> full [B, L, C] output."""
    out = np.empty((B, L, C), dtype=np.float32)
    for b in range(B):
        o0 = results[2 * b]["outT"]          # [C, L], true l order
        o1 = results[2 * b + 1]["outT"]      # [C, L], l rotated by MH
        o1 = np.concatenate([o1[:, MH:], o1[:, :MH]], axis=1)
        out[b] = (o0 + o1).T
    return out


def kernel(**inputs):
    from concourse.bass_utils import run_bass_kernel_spmd

    nc = _get_nc("full")
    in_maps = _prep_core_inputs(**inputs)
    res = run_bass_kernel_spmd(nc, in_maps, core_ids=list(range(NCORES)))
    return _combine(res.results)


if __name__ == "__main__":
    rng = np.random.default_rng(0)
    ins = {
        "x": rng.standard_normal((B, L, C), dtype=np.float32),
        "rel_h": rng.standard_normal((1, Cr, 64, 1), dtype=np.float32),
        "rel_w": rng.standard_normal((1, Cr, 1, 64), dtype=np.float32),
        "Wq": rng.standard_normal((C, Cr), dtype=np.float32) * 0.02,
        "bq": np.zeros(Cr, np.float32),
        "Wk": rng.standard_normal((C, Cr), dtype=np.float32) * 0.02,
        "bk": np.zeros(Cr, np.float32),
        "Wv": rng.standard_normal((C, C), dtype=np.float32) * 0.02,
        "bv": np.zeros(C, np.float32),
    }
    out = kernel(**ins)
    print(out.shape, out.dtype)


# revision 35
# speedup vs baseline: 1.4380x; 1.4380x over previous
"""Trainium2 Bass kernel for nn_Att_61881888801149 (sparse_attention).

Math (per batch b):
    q = x @ Wq + bq                  [L, Cr]
    k = x @ Wk + bk                  [L, Cr]
    v = x @ Wv + bv                  [L, C]
    pos = (rel_h + rel_w).reshape(Cr, L)
    S = q @ (k^T + pos)              [L, L]   (queries l, keys m)
    attn = softmax(S, axis=0)        (normalized over the QUERY axis l)
    out = attn @ v                   [L, C]

Because the softmax axis (l) is orthogonal to the bmm contraction axis (m):
    out[l, c] = sum_m  E[l, m] * v[m, c] / colsum[m]
with E = exp(S) (no max subtraction needed - scores are small), and
colsum[m] = sum_l E[l, m].

Sharding: 8 cores = 4 batches x 2 key-halves (m in [0,2048) or [2048,4096)).
Host sums the two partial outputs per batch.  SPMD trick: the host rotates
xT's columns per-core so each core's m-half is always columns 0:2048; the
output columns (l, also rotated) are un-rotated on the host.

On-core layout: everything is computed transposed:
    qT  [Cr, L]  = Wq^T @ xT + bq
    kpT [Cr, M]  = Wk^T @ xTm + (pos + bk)         (pos+bk folded on host)
    vb  [M, C]   = xTm^T @ Wv + bv (rank-1 ones matmul for the bias)
    ST  [M, L]   = kpT^T @ qT      -> exp (ACT, fused colsum accumulation)
    E   [M, L]   bf16, resident in SBUF (16MB)
    vbw [M, C]   = vb * (1/colsum) per row, bf16
    outT[C, L]   = vbw^T @ E       (PSUM accumulation over m-blocks)
"""

import sys

for _p in ("/opt/trn_rl_repo", "/root/.axon_site/_ro/trn_rl_repo"):
    if _p not in sys.path:
        sys.path.append(_p)

import numpy as np

B, L, C, Cr = 4, 4096, 256, 32
MH = L // 2  # per-core key-half size (2048)
NCORES = 8

_CACHE = {}


def build_nc(L=L, C=C, Cr=Cr, M=MH):
    import concourse.bass as bass
    import concourse.tile as tile
    from concourse import mybir
    from concourse.tile_rust import add_dep_helper

    FP32 = mybir.dt.float32
    FP16 = mybir.dt.float16
    Exp = mybir.ActivationFunctionType.Exp
    # E is stored as fp16 exp(S - OFF).  The offset cancels exactly in
    # out = E' @ (v / colsum(E')) and keeps exp(S) inside fp16 range:
    # real-data S in [-19, 19.44], colmax in [3.6, 19.44] -> E' <= e^7.9,
    # vbw' <= |v| * e^{OFF - colmax_min} ~ 4e3, both with >= 16x margin.
    EXP_OFF = 11.5

    assert C == 256 and Cr == 32
    assert L % 1024 == 0 and M % 512 == 0
    NMB = M // 128          # m-blocks per core
    NLG = L // 512          # l-groups for phase 2
    SG = 1024               # phase-1 ACT exp chunk width
    NSG = L // SG           # stats groups per m-block

    # xw (fp16) columns: wq0 0:32 | wq1 32:64 | wk0 64:96 | wk1 96:128 |
    # wv0 128:384 | wv1 384:640 | bq(row0) 640:672 | bv(row0) 672:928 |
    # ones(row0) 928:1440
    nc = bass.Bass()
    xt_d = nc.dram_tensor("xt", [128, 2 * L], FP16, kind="ExternalInput")
    xw_d = nc.dram_tensor("xw", [128, 1440], FP16, kind="ExternalInput")
    pos_d = nc.dram_tensor("pos", [Cr, M], FP32, kind="ExternalInput")
    outT_d = nc.dram_tensor("outT", [C, L], FP32, kind="ExternalOutput")

    with tile.TileContext(nc) as tc:
        with (
            tc.tile_pool(name="persist", bufs=1) as persist,
            tc.tile_pool(name="psum", bufs=1, space="PSUM") as psum,
        ):
            qT = persist.tile([Cr, L], FP16)
            kpT = persist.tile([Cr, M], FP16)
            vb = persist.tile([128, NMB, C], FP16)
            vbw = persist.tile([128, NMB, C], FP16)
            stats = persist.tile([128, NMB, NSG], FP32)
            colsum = persist.tile([128, NMB], FP32)
            wrec = persist.tile([128, NMB], FP32)
            expoff = persist.tile([128, 1], FP32)
            nc.vector.memset(expoff[:], -EXP_OFF)
            # tiny fp16 tile for Ldweights "carrier" instructions: a PE op
            # that takes the cross-engine WAR wait of a PSUM slot being
            # re-opened, so the slot-opening Matmult (1-sem-wait ISA
            # budget) only carries its same-engine bank WAW wait.
            wdum = persist.tile([1, 1], FP16)
            nc.vector.memset(wdum[:], 0.0)

            def carrier(dep):
                if dep is None:
                    return None
                c = nc.tensor.ldweights(wdum[:])
                add_dep_helper(c.ins, dep.ins, sync=True,
                               reason="psum slot WAR carrier")
                return c

            def anchor(mm, c):
                if c is not None:
                    add_dep_helper(mm.ins, c.ins, sync=False,
                                   reason="carrier anchor")
                return mm

            with tc.tile_pool(name="epool", bufs=1) as epool:
                E = epool.tile([128, NMB, L], FP16)

                # ---- prolog pool stays open through phase 1 so vb matmul
                # emission can interleave with the ST/exp stream ----
                with tc.tile_pool(name="prolog", bufs=1) as pp:
                    xw = pp.tile([128, 1440], FP16)
                    nc.sync.dma_start(xw[:], xw_d[:])
                    pos = pp.tile([Cr, M], FP32)
                    nc.gpsimd.dma_start(pos[:], pos_d[:])
                    xt = pp.tile([128, 2, L], FP16)
                    for j in range(L // 1024):
                        for half in range(2):
                            c0 = half * L + j * 1024
                            eng = nc.sync if half == 0 else nc.gpsimd
                            eng.dma_start(xt[:, half, j * 1024:(j + 1) * 1024],
                                          xt_d[:, c0:c0 + 1024])
                    dvew = pp.tile([1, 1], FP32)
                    nc.vector.tensor_copy(dvew[:], pos[0:1, 0:1])

                    wq0, wq1 = xw[:, 0:32], xw[:, 32:64]
                    wk0, wk1 = xw[:, 64:96], xw[:, 96:128]
                    wv0, wv1 = xw[:, 128:384], xw[:, 384:640]
                    bq = xw[0:1, 640:672]
                    bv = xw[0:1, 672:928]
                    ones = xw[0:1, 928:1440]

                    hist_q = [None, None]
                    hist_v = [None, None, None, None]
                    hist_st = [None, None]

                    # qT = Wq^T @ xT + bq (bias via rank-1 ones matmul)
                    for j in range(L // 512):
                        sl = slice(j * 512, (j + 1) * 512)
                        cr_ = carrier(hist_q[j % 2])
                        psq_t = psum.tile([128, SG], FP32, tag="st", bufs=2)
                        psq = psq_t[0:Cr, 0:512]
                        anchor(nc.tensor.matmul(psq[:], wq0, xt[:, 0, sl],
                                                start=True, stop=False), cr_)
                        nc.tensor.matmul(psq[:], wq1, xt[:, 1, sl],
                                         start=False, stop=False)
                        nc.tensor.matmul(psq[:], bq, ones[0:1, 0:512],
                                         start=False, stop=True)
                        hist_q[j % 2] = nc.scalar.copy(qT[:, sl], psq[:])

                    # kpT = Wk^T @ xTm + pos (pos already includes bk)
                    for j in range(M // 512):
                        sl = slice(j * 512, (j + 1) * 512)
                        cr_ = carrier(hist_q[j % 2])
                        psk_t = psum.tile([128, SG], FP32, tag="st", bufs=2)
                        psk = psk_t[0:Cr, 0:512]
                        anchor(nc.tensor.matmul(psk[:], wk0, xt[:, 0, sl],
                                                start=True, stop=False), cr_)
                        nc.tensor.matmul(psk[:], wk1, xt[:, 1, sl],
                                         start=False, stop=True)
                        hist_q[j % 2] = nc.vector.tensor_add(
                            kpT[:, sl], psk[:], pos[:, sl])

                    # ---- phase 1 (vb groups emitted after 4 m-blocks so
                    # the exp stream starts immediately; vb[mb] is only
                    # needed after mb's colsum) ----
                    last_vb = {}

                    def stats_tail(mb):
                        nc.vector.reduce_sum(colsum[:, mb:mb + 1],
                                             stats[:, mb, :],
                                             axis=mybir.AxisListType.X)
                        nc.vector.reciprocal(wrec[:, mb:mb + 1],
                                             colsum[:, mb:mb + 1])
                        nc.vector.tensor_scalar_mul(
                            vbw[:, mb, :], vb[:, mb, :], wrec[:, mb:mb + 1])

                    VB_AT = min(4, NMB - 1)
                    for mb in range(NMB):
                        if mb == VB_AT:
                            for vmb in range(NMB):
                                msl = slice(vmb * 128, (vmb + 1) * 128)
                                cr_ = carrier(hist_v[vmb % 4])
                                psv_t = psum.tile([128, 512], FP32,
                                                  tag="po", bufs=4)
                                psv = psv_t[:, 0:C]
                                anchor(nc.tensor.matmul(
                                    psv[:], xt[:, 0, msl], wv0,
                                    start=True, stop=False), cr_)
                                nc.tensor.matmul(psv[:], xt[:, 1, msl], wv1,
                                                 start=False, stop=False)
                                nc.tensor.matmul(psv[:], ones[0:1, 0:128],
                                                 bv, start=False, stop=True)
                                lvb = nc.vector.tensor_copy(
                                    vb[:, vmb, :], psv[:])
                                hist_v[vmb % 4] = lvb
                                last_vb[vmb] = lvb
                            # emit the deferred stats tails now that vb exists
                            for pmb in range(VB_AT):
                                stats_tail(pmb)
                        kp_sl = kpT[:, mb * 128:(mb + 1) * 128]
                        for g in range(NSG):
                            idx = mb * NSG + g
                            cr_ = carrier(hist_st[idx % 2])
                            ps = psum.tile([128, SG], FP32, tag="st", bufs=2)
                            last = None
                            for j in range(SG // 512):
                                lsl = slice(g * SG + j * 512,
                                            g * SG + (j + 1) * 512)
                                last = nc.tensor.matmul(
                                    ps[:, j * 512:(j + 1) * 512],
                                    kp_sl, qT[:, lsl], start=True, stop=True)
                                if j == 0:
                                    anchor(last, cr_)
                            last_exp = nc.scalar.activation(
                                E[:, mb, g * SG:(g + 1) * SG], ps[:], Exp,
                                bias=expoff[:],
                                accum_out=stats[:, mb, g:g + 1])
                            hist_st[idx % 2] = last_exp
                        if mb >= VB_AT:
                            stats_tail(mb)

                # ---- phase 2 (prolog closed; stage pools reuse its zone).
                # Split m-accumulation: partA = mb 0..HM-1 closes mid
                # phase-1; partB quarters close at 3/4 and at the end. ----
                kgrp = 16
                HM = NMB // 2
                with (
                    tc.tile_pool(name="stagea", bufs=16) as stagea,
                    tc.tile_pool(name="stage", bufs=4) as stage,
                ):
                    soas = []
                    for lg in range(NLG):
                        lsl = slice(lg * 512, (lg + 1) * 512)
                        for ch in range(C // 128):
                            cr_ = carrier(hist_v[kgrp % 4])
                            poa = psum.tile([128, 512], FP32, tag="po", bufs=4)
                            last = None
                            for mb in range(HM):
                                last = nc.tensor.matmul(
                                    poa[:],
                                    vbw[:, mb, ch * 128:(ch + 1) * 128],
                                    E[:, mb, lsl],
                                    start=(mb == 0), stop=(mb == HM - 1))
                                if mb == 0:
                                    anchor(last, cr_)
                            soa = stagea.tile([128, 512], FP16, tag="soa",
                                              name=f"soa_{lg}_{ch}")
                            hist_v[kgrp % 4] = nc.vector.tensor_copy(
                                soa[:], poa[:])
                            soas.append(soa)
                            kgrp += 1

                    for q, (m0, m1) in enumerate([(HM, HM + HM // 2),
                                                  (HM + HM // 2, NMB)]):
                        for lg in range(NLG):
                            lsl = slice(lg * 512, (lg + 1) * 512)
                            for ch in range(C // 128):
                                cr_ = carrier(hist_v[kgrp % 4])
                                pob = psum.tile([128, 512], FP32, tag="po",
                                                bufs=4)
                                last = None
                                for mb in range(m0, m1):
                                    last = nc.tensor.matmul(
                                        pob[:],
                                        vbw[:, mb, ch * 128:(ch + 1) * 128],
                                        E[:, mb, lsl],
                                        start=(mb == m0), stop=(mb == m1 - 1))
                                    if mb == m0:
                                        anchor(last, cr_)
                                soa = soas[lg * 2 + ch]
                                if q == 0:
                                    hist_v[kgrp % 4] = nc.vector.tensor_add(
                                        soa[:], pob[:], soa[:])
                                    kgrp += 1
                                else:
                                    so = stage.tile([128, 512], FP32,
                                                    tag="so")
                                    hist_v[kgrp % 4] = nc.vector.tensor_add(
                                        so[:], pob[:], soa[:])
                                    kgrp += 1
                                    nc.sync.dma_start(
                                        outT_d[ch * 128:(ch + 1) * 128, lsl],
                                        so[:])

    return nc


def _fixup_waits(nc):
    """Walrus codegen on this toolchain allows only ~1 semaphore wait per
    TPB instruction (2 for DMACopy).  Hoist excess waits into standalone
    single-wait EventSemaphore instructions inserted just before the
    over-budget instruction on the same engine (same-stream ordering makes
    this semantics-preserving)."""
    from concourse import mybir

    budget_by_type = {}
    n = 0
    for fn in nc.m.functions:
        for blk in fn.blocks:
            insts = blk.instructions
            i = 0
            while i < len(insts):
                inst = insts[i]
                si = getattr(inst, "sync_info", None)
                if si is None:
                    i += 1
                    continue
                waits = list(si.on_wait)
                budget = budget_by_type.get(type(inst).__name__, 1)
                if len(waits) <= budget:
                    i += 1
                    continue
                extra, keep = waits[:-budget], waits[-budget:]
                for w in extra:
                    es = mybir.InstEventSemaphore(
                        name=f"es_waitfix_{n}", ins=[], outs=[])
                    n += 1
                    es.engine = inst.engine
                    es.sync_info = mybir.SyncInfo(on_wait=[w], on_update=[])
                    insts.insert(i, es)
                    i += 1
                inst.sync_info = mybir.SyncInfo(
                    on_wait=keep, on_update=list(si.on_update))
                i += 1


def _build_and_fix(**kw):
    nc = build_nc(**kw)
    _fixup_waits(nc)
    return nc


def _get_nc(key, **kw):
    if key not in _CACHE:
        _CACHE[key] = _build_and_fix(**kw)
    return _CACHE[key]


def _prep_core_inputs(x, rel_h, rel_w, Wq, bq, Wk, bk, Wv, bv):
    """Build the 8 per-core input maps (host-side sharding / layout prep)."""
    x = np.asarray(x, dtype=np.float32)
    pos = (np.asarray(rel_h, np.float32) + np.asarray(rel_w, np.float32))
    pos = pos.reshape(Cr, L) + np.asarray(bk, np.float32).reshape(Cr, 1)
    xw = np.zeros((128, 1440), np.float16)
    xw[:, 0:32] = np.asarray(Wq, np.float32)[0:128]
    xw[:, 32:64] = np.asarray(Wq, np.float32)[128:256]
    xw[:, 64:96] = np.asarray(Wk, np.float32)[0:128]
    xw[:, 96:128] = np.asarray(Wk, np.float32)[128:256]
    xw[:, 128:384] = np.asarray(Wv, np.float32)[0:128]
    xw[:, 384:640] = np.asarray(Wv, np.float32)[128:256]
    xw[0, 640:672] = np.asarray(bq, np.float32).ravel()
    xw[0, 672:928] = np.asarray(bv, np.float32).ravel()
    xw[0, 928:1440] = 1.0

    in_maps = []
    for i in range(NCORES):
        b, h = divmod(i, 2)
        xT = x[b].T.astype(np.float16)  # [C, L]
        if h == 1:
            xT = np.concatenate([xT[:, MH:], xT[:, :MH]], axis=1)
        xtc = np.ascontiguousarray(
            np.concatenate([xT[0:128], xT[128:256]], axis=1))
        posh = np.ascontiguousarray(pos[:, h * MH:(h + 1) * MH])
        in_maps.append({"xt": xtc, "xw": xw, "pos": posh})
    return in_maps


def _combine(results):
    """results: list of 8 out_maps -> full [B, L, C] output."""
    out = np.empty((B, L, C), dtype=np.float32)
    for b in range(B):
        o0 = results[2 * b]["outT"]          # [C, L], true l order
        o1 = results[2 * b + 1]["outT"]      # [C, L], l rotated by MH
        o1 = np.concatenate([o1[:, MH:], o1[:, :MH]], axis=1)
        out[b] = (o0 + o1).T
    return out


def kernel(**inputs):
    from concourse.bass_utils import run_bass_kernel_spmd

    nc = _get_nc("full")
    in_maps = _prep_core_inputs(**inputs)
    res = run_bass_kernel_spmd(nc, in_maps, core_ids=list(range(NCORES)))
    return _combine(res.results)


if __name__ == "__main__":
    rng = np.random.default_rng(0)
    ins = {
        "x": rng.standard_normal((B, L, C), dtype=np.float32),
        "rel_h": rng.standard_normal((1, Cr, 64, 1), dtype=np.float32),
        "rel_w": rng.standard_normal((1, Cr, 1, 64), dtype=np.float32),
        "Wq": rng.standard_normal((C, Cr), dtype=np.float32) * 0.02,
        "bq": np.zeros(Cr, np.float32),
        "Wk": rng.standard_normal((C, Cr), dtype=np.float32) * 0.02,
        "bk": np.zeros(Cr, np.float32),
        "Wv": rng.standard_normal((C, C), dtype=np.float32) * 0.02,
        "bv": np.zeros(C, np.float32),
    }
    out = kernel(**ins)
    print(out.shape, out.dtype)


# revision 40
# speedup vs baseline: 1.4731x; 1.0244x over previous
"""Trainium2 Bass kernel for nn_Att_61881888801149 (sparse_attention).

Math (per batch b):
    q = x @ Wq + bq                  [L, Cr]
    k = x @ Wk + bk                  [L, Cr]
    v = x @ Wv + bv                  [L, C]
    pos = (rel_h + rel_w).reshape(Cr, L)
    S = q @ (k^T + pos)              [L, L]   (queries l, keys m)
    attn = softmax(S, axis=0)        (normalized over the QUERY axis l)
    out = attn @ v                   [L, C]

Because the softmax axis (l) is orthogonal to the bmm contraction axis (m):
    out[l, c] = sum_m  E[l, m] * v[m, c] / colsum[m]
with E = exp(S) (no max subtraction needed - scores are small), and
colsum[m] = sum_l E[l, m].

Sharding: 8 cores = 4 batches x 2 key-halves (m in [0,2048) or [2048,4096)).
Host sums the two partial outputs per batch.  SPMD trick: the host rotates
xT's columns per-core so each core's m-half is always columns 0:2048; the
output columns (l, also rotated) are un-rotated on the host.

On-core layout: everything is computed transposed:
    qT  [Cr, L]  = Wq^T @ xT + bq
    kpT [Cr, M]  = Wk^T @ xTm + (pos + bk)         (pos+bk folded on host)
    vb  [M, C]   = xTm^T @ Wv + bv (rank-1 ones matmul for the bias)
    ST  [M, L]   = kpT^T @ qT      -> exp (ACT, fused colsum accumulation)
    E   [M, L]   bf16, resident in SBUF (16MB)
    vbw [M, C]   = vb * (1/colsum) per row, bf16
    outT[C, L]   = vbw^T @ E       (PSUM accumulation over m-blocks)
"""

import sys

for _p in ("/opt/trn_rl_repo", "/root/.axon_site/_ro/trn_rl_repo"):
    if _p not in sys.path:
        sys.path.append(_p)

import numpy as np

B, L, C, Cr = 4, 4096, 256, 32
MH = L // 2  # per-core key-half size (2048)
NCORES = 8

_CACHE = {}


def build_nc(L=L, C=C, Cr=Cr, M=MH):
    import concourse.bass as bass
    import concourse.tile as tile
    from concourse import mybir
    from concourse.tile_rust import add_dep_helper

    FP32 = mybir.dt.float32
    FP16 = mybir.dt.float16
    Exp = mybir.ActivationFunctionType.Exp
    # E is stored as fp16 exp(S - OFF).  The offset cancels exactly in
    # out = E' @ (v / colsum(E')) and keeps exp(S) inside fp16 range:
    # real-data S in [-19, 19.44], colmax in [3.6, 19.44] -> E' <= e^7.9,
    # vbw' <= |v| * e^{OFF - colmax_min} ~ 4e3, both with >= 16x margin.
    EXP_OFF = 11.5

    assert C == 256 and Cr == 32
    assert L % 1024 == 0 and M % 512 == 0
    NMB = M // 128          # m-blocks per core
    NLG = L // 512          # l-groups for phase 2
    SG = 1024               # phase-1 ACT exp chunk width
    NSG = L // SG           # stats groups per m-block

    # xw (fp16) columns: wq0 0:32 | wq1 32:64 | wk0 64:96 | wk1 96:128 |
    # wv0 128:384 | wv1 384:640 | bq(row0) 640:672 | bv(row0) 672:928 |
    # ones(row0) 928:1440
    nc = bass.Bass()
    xt_d = nc.dram_tensor("xt", [128, 2 * L], FP16, kind="ExternalInput")
    xw_d = nc.dram_tensor("xw", [128, 1440], FP16, kind="ExternalInput")
    pos_d = nc.dram_tensor("pos", [Cr, M], FP32, kind="ExternalInput")
    outT_d = nc.dram_tensor("outT", [C, L], FP32, kind="ExternalOutput")

    with tile.TileContext(nc) as tc:
        with (
            tc.tile_pool(name="persist", bufs=1) as persist,
            tc.tile_pool(name="psum", bufs=1, space="PSUM") as psum,
        ):
            qT = persist.tile([Cr, L], FP16)
            kpT = persist.tile([Cr, M], FP16)
            vb = persist.tile([128, NMB, C], FP16)
            vbw = persist.tile([128, NMB, C], FP16)
            stats = persist.tile([128, NMB, NSG], FP32)
            colsum = persist.tile([128, NMB], FP32)
            wrec = persist.tile([128, NMB], FP32)
            expoff = persist.tile([128, 1], FP32)
            nc.vector.memset(expoff[:], -EXP_OFF)
            # tiny fp16 tile for Ldweights "carrier" instructions: a PE op
            # that takes the cross-engine WAR wait of a PSUM slot being
            # re-opened, so the slot-opening Matmult (1-sem-wait ISA
            # budget) only carries its same-engine bank WAW wait.
            wdum = persist.tile([1, 1], FP16)
            nc.vector.memset(wdum[:], 0.0)

            def carrier(dep):
                if dep is None:
                    return None
                c = nc.tensor.ldweights(wdum[:])
                add_dep_helper(c.ins, dep.ins, sync=True,
                               reason="psum slot WAR carrier")
                return c

            def anchor(mm, c):
                if c is not None:
                    add_dep_helper(mm.ins, c.ins, sync=False,
                                   reason="carrier anchor")
                return mm

            with tc.tile_pool(name="epool", bufs=1) as epool:
                E = epool.tile([128, NMB, L], FP16)

                # ---- prolog pool stays open through phase 1 so vb matmul
                # emission can interleave with the ST/exp stream ----
                with tc.tile_pool(name="prolog", bufs=1) as pp:
                    xw = pp.tile([128, 1440], FP16)
                    nc.sync.dma_start(xw[:], xw_d[:])
                    pos = pp.tile([Cr, M], FP32)
                    nc.gpsimd.dma_start(pos[:], pos_d[:])
                    xt = pp.tile([128, 2, L], FP16)
                    for j in range(L // 1024):
                        for half in range(2):
                            c0 = half * L + j * 1024
                            eng = nc.sync if half == 0 else nc.gpsimd
                            eng.dma_start(xt[:, half, j * 1024:(j + 1) * 1024],
                                          xt_d[:, c0:c0 + 1024])
                    dvew = pp.tile([1, 1], FP32)
                    nc.vector.tensor_copy(dvew[:], pos[0:1, 0:1])

                    wq0, wq1 = xw[:, 0:32], xw[:, 32:64]
                    wk0, wk1 = xw[:, 64:96], xw[:, 96:128]
                    wv0, wv1 = xw[:, 128:384], xw[:, 384:640]
                    bq = xw[0:1, 640:672]
                    bv = xw[0:1, 672:928]
                    ones = xw[0:1, 928:1440]

                    hist_po = [None, None, None, None]
                    hist_st = [None, None]
                    kidx = [0]

                    def po_tile(name):
                        cr_ = carrier(hist_po[kidx[0] % 4])
                        t = psum.tile([128, 512], FP32, tag="po", bufs=4,
                                      name=name)
                        return t, cr_

                    def po_done(reader):
                        hist_po[kidx[0] % 4] = reader
                        kidx[0] += 1

                    # qT / kpT groups are emitted lazily inside the
                    # phase-1 loop so the exp stream starts as soon as the
                    # first chunks are ready (PE executes in queue order).
                    qdone = set()
                    kdone = set()

                    def need_q(j):
                        if j in qdone:
                            return
                        qdone.add(j)
                        sl = slice(j * 512, (j + 1) * 512)
                        psq_t, cr_ = po_tile(f"psq_{j}")
                        psq = psq_t[0:Cr, 0:512]
                        anchor(nc.tensor.matmul(psq[:], wq0, xt[:, 0, sl],
                                                start=True, stop=False), cr_)
                        nc.tensor.matmul(psq[:], wq1, xt[:, 1, sl],
                                         start=False, stop=False)
                        nc.tensor.matmul(psq[:], bq, ones[0:1, 0:512],
                                         start=False, stop=True)
                        po_done(nc.scalar.copy(qT[:, sl], psq[:]))

                    def need_k(j):
                        if j in kdone:
                            return
                        kdone.add(j)
                        sl = slice(j * 512, (j + 1) * 512)
                        psk_t, cr_ = po_tile(f"psk_{j}")
                        psk = psk_t[0:Cr, 0:512]
                        anchor(nc.tensor.matmul(psk[:], wk0, xt[:, 0, sl],
                                                start=True, stop=False), cr_)
                        nc.tensor.matmul(psk[:], wk1, xt[:, 1, sl],
                                         start=False, stop=True)
                        po_done(nc.vector.tensor_add(
                            kpT[:, sl], psk[:], pos[:, sl]))

                    # ---- phase 1 (vb groups emitted after 4 m-blocks so
                    # the exp stream starts immediately; vb[mb] is only
                    # needed after mb's colsum) ----
                    last_vb = {}

                    def stats_tail(mb):
                        nc.vector.reduce_sum(colsum[:, mb:mb + 1],
                                             stats[:, mb, :],
                                             axis=mybir.AxisListType.X)
                        nc.vector.reciprocal(wrec[:, mb:mb + 1],
                                             colsum[:, mb:mb + 1])
                        nc.vector.tensor_scalar_mul(
                            vbw[:, mb, :], vb[:, mb, :], wrec[:, mb:mb + 1])

                    VB_AT = min(4, NMB - 1)
                    for mb in range(NMB):
                        if mb % 4 == 0:
                            need_k(mb // 4)
                        if mb == VB_AT:
                            for vmb in range(NMB):
                                msl = slice(vmb * 128, (vmb + 1) * 128)
                                psv_t, cr_ = po_tile(f"psv_{vmb}")
                                psv = psv_t[:, 0:C]
                                anchor(nc.tensor.matmul(
                                    psv[:], xt[:, 0, msl], wv0,
                                    start=True, stop=False), cr_)
                                nc.tensor.matmul(psv[:], xt[:, 1, msl], wv1,
                                                 start=False, stop=False)
                                nc.tensor.matmul(psv[:], ones[0:1, 0:128],
                                                 bv, start=False, stop=True)
                                lvb = nc.vector.tensor_copy(
                                    vb[:, vmb, :], psv[:])
                                po_done(lvb)
                                last_vb[vmb] = lvb
                            # emit the deferred stats tails now that vb exists
                            for pmb in range(VB_AT):
                                stats_tail(pmb)
                        kp_sl = kpT[:, mb * 128:(mb + 1) * 128]
                        for g in range(NSG):
                            if mb == 0:
                                need_q(2 * g)
                                need_q(2 * g + 1)
                            idx = mb * NSG + g
                            cr_ = carrier(hist_st[idx % 2])
                            ps = psum.tile([128, SG], FP32, tag="st", bufs=2)
                            last = None
                            for j in range(SG // 512):
                                lsl = slice(g * SG + j * 512,
                                            g * SG + (j + 1) * 512)
                                last = nc.tensor.matmul(
                                    ps[:, j * 512:(j + 1) * 512],
                                    kp_sl, qT[:, lsl], start=True, stop=True)
                                if j == 0:
                                    anchor(last, cr_)
                            last_exp = nc.scalar.activation(
                                E[:, mb, g * SG:(g + 1) * SG], ps[:], Exp,
                                bias=expoff[:],
                                accum_out=stats[:, mb, g:g + 1])
                            hist_st[idx % 2] = last_exp
                        if mb >= VB_AT:
                            stats_tail(mb)

                # ---- phase 2 (prolog closed; stage pools reuse its zone).
                # Split m-accumulation: partA = mb 0..HM-1 closes mid
                # phase-1; partB quarters close at 3/4 and at the end. ----
                HM = NMB // 2
                with (
                    tc.tile_pool(name="stagea", bufs=16) as stagea,
                    tc.tile_pool(name="stage", bufs=4) as stage,
                ):
                    soas = []
                    for lg in range(NLG):
                        lsl = slice(lg * 512, (lg + 1) * 512)
                        for ch in range(C // 128):
                            poa, cr_ = po_tile(f"poa_{lg}_{ch}")
                            last = None
                            for mb in range(HM):
                                last = nc.tensor.matmul(
                                    poa[:],
                                    vbw[:, mb, ch * 128:(ch + 1) * 128],
                                    E[:, mb, lsl],
                                    start=(mb == 0), stop=(mb == HM - 1))
                                if mb == 0:
                                    anchor(last, cr_)
                            soa = stagea.tile([128, 512], FP16, tag="soa",
                                              name=f"soa_{lg}_{ch}")
                            po_done(nc.vector.tensor_copy(soa[:], poa[:]))
                            soas.append(soa)

                    QR = NMB // 4
                    splits = [(HM, HM + QR), (HM + QR, NMB)]
                    splits = [(a, b) for a, b in splits if b > a]
                    for q, (m0, m1) in enumerate(splits):
                        for lg in range(NLG):
                            lsl = slice(lg * 512, (lg + 1) * 512)
                            for ch in range(C // 128):
                                pob, cr_ = po_tile(f"pob_{q}_{lg}_{ch}")
                                last = None
                                for mb in range(m0, m1):
                                    last = nc.tensor.matmul(
                                        pob[:],
                                        vbw[:, mb, ch * 128:(ch + 1) * 128],
                                        E[:, mb, lsl],
                                        start=(mb == m0), stop=(mb == m1 - 1))
                                    if mb == m0:
                                        anchor(last, cr_)
                                soa = soas[lg * 2 + ch]
                                if q < len(splits) - 1:
                                    po_done(nc.vector.tensor_add(
                                        soa[:], pob[:], soa[:]))
                                else:
                                    so = stage.tile([128, 512], FP32,
                                                    tag="so")
                                    po_done(nc.vector.tensor_add(
                                        so[:], pob[:], soa[:]))
                                    nc.sync.dma_start(
                                        outT_d[ch * 128:(ch + 1) * 128, lsl],
                                        so[:])

    return nc


def _fixup_waits(nc):
    """Walrus codegen on this toolchain allows only ~1 semaphore wait per
    TPB instruction (2 for DMACopy).  Hoist excess waits into standalone
    single-wait EventSemaphore instructions inserted just before the
    over-budget instruction on the same engine (same-stream ordering makes
    this semantics-preserving)."""
    from concourse import mybir

    budget_by_type = {}
    n = 0
    for fn in nc.m.functions:
        for blk in fn.blocks:
            insts = blk.instructions
            i = 0
            while i < len(insts):
                inst = insts[i]
                si = getattr(inst, "sync_info", None)
                if si is None:
                    i += 1
                    continue
                waits = list(si.on_wait)
                budget = budget_by_type.get(type(inst).__name__, 1)
                if len(waits) <= budget:
                    i += 1
                    continue
                extra, keep = waits[:-budget], waits[-budget:]
                for w in extra:
                    es = mybir.InstEventSemaphore(
                        name=f"es_waitfix_{n}", ins=[], outs=[])
                    n += 1
                    es.engine = inst.engine
                    es.sync_info = mybir.SyncInfo(on_wait=[w], on_update=[])
                    insts.insert(i, es)
                    i += 1
                inst.sync_info = mybir.SyncInfo(
                    on_wait=keep, on_update=list(si.on_update))
                i += 1


def _build_and_fix(**kw):
    nc = build_nc(**kw)
    _fixup_waits(nc)
    return nc


def _get_nc(key, **kw):
    if key not in _CACHE:
        _CACHE[key] = _build_and_fix(**kw)
    return _CACHE[key]


def _prep_core_inputs(x, rel_h, rel_w, Wq, bq, Wk, bk, Wv, bv):
    """Build the 8 per-core input maps (host-side sharding / layout prep)."""
    x = np.asarray(x, dtype=np.float32)
    pos = (np.asarray(rel_h, np.float32) + np.asarray(rel_w, np.float32))
    pos = pos.reshape(Cr, L) + np.asarray(bk, np.float32).reshape(Cr, 1)
    xw = np.zeros((128, 1440), np.float16)
    xw[:, 0:32] = np.asarray(Wq, np.float32)[0:128]
    xw[:, 32:64] = np.asarray(Wq, np.float32)[128:256]
    xw[:, 64:96] = np.asarray(Wk, np.float32)[0:128]
    xw[:, 96:128] = np.asarray(Wk, np.float32)[128:256]
    xw[:, 128:384] = np.asarray(Wv, np.float32)[0:128]
    xw[:, 384:640] = np.asarray(Wv, np.float32)[128:256]
    xw[0, 640:672] = np.asarray(bq, np.float32).ravel()
    xw[0, 672:928] = np.asarray(bv, np.float32).ravel()
    xw[0, 928:1440] = 1.0

    in_maps = []
    for i in range(NCORES):
        b, h = divmod(i, 2)
        xT = x[b].T.astype(np.float16)  # [C, L]
        if h == 1:
            xT = np.concatenate([xT[:, MH:], xT[:, :MH]], axis=1)
        xtc = np.ascontiguousarray(
            np.concatenate([xT[0:128], xT[128:256]], axis=1))
        posh = np.ascontiguousarray(pos[:, h * MH:(h + 1) * MH])
        in_maps.append({"xt": xtc, "xw": xw, "pos": posh})
    return in_maps


def _combine(results):
    """results: list of 8 out_maps -> full [B, L, C] output."""
    out = np.empty((B, L, C), dtype=np.float32)
    for b in range(B):
        o0 = results[2 * b]["outT"]          # [C, L], true l order
        o1 = results[2 * b + 1]["outT"]      # [C, L], l rotated by MH
        o1 = np.concatenate([o1[:, MH:], o1[:, :MH]], axis=1)
        out[b] = (o0 + o1).T
    return out


def kernel(**inputs):
    from concourse.bass_utils import run_bass_kernel_spmd

    nc = _get_nc("full")
    in_maps = _prep_core_inputs(**inputs)
    res = run_bass_kernel_spmd(nc, in_maps, core_ids=list(range(NCORES)))
    return _combine(res.results)


if __name__ == "__main__":
    rng = np.random.default_rng(0)
    ins = {
        "x": rng.standard_normal((B, L, C), dtype=np.float32),
        "rel_h": rng.standard_normal((1, Cr, 64, 1), dtype=np.float32),
        "rel_w": rng.standard_normal((1, Cr, 1, 64), dtype=np.float32),
        "Wq": rng.standard_normal((C, Cr), dtype=np.float32) * 0.02,
        "bq": np.zeros(Cr, np.float32),
        "Wk": rng.standard_normal((C, Cr), dtype=np.float32) * 0.02,
        "bk": np.zeros(Cr, np.float32),
        "Wv": rng.standard_normal((C, C), dtype=np.float32) * 0.02,
        "bv": np.zeros(C, np.float32),
    }
    out = kernel(**ins)
    print(out.shape, out.dtype)


# revision 44
# speedup vs baseline: 1.5221x; 1.0333x over previous
"""Trainium2 Bass kernel for nn_Att_61881888801149 (sparse_attention).

Math (per batch b):
    q = x @ Wq + bq                  [L, Cr]
    k = x @ Wk + bk                  [L, Cr]
    v = x @ Wv + bv                  [L, C]
    pos = (rel_h + rel_w).reshape(Cr, L)
    S = q @ (k^T + pos)              [L, L]   (queries l, keys m)
    attn = softmax(S, axis=0)        (normalized over the QUERY axis l)
    out = attn @ v                   [L, C]

Because the softmax axis (l) is orthogonal to the bmm contraction axis (m):
    out[l, c] = sum_m  E[l, m] * v[m, c] / colsum[m]
with E = exp(S) (no max subtraction needed - scores are small), and
colsum[m] = sum_l E[l, m].

Sharding: 8 cores = 4 batches x 2 key-halves (m in [0,2048) or [2048,4096)).
Host sums the two partial outputs per batch.  SPMD trick: the host rotates
xT's columns per-core so each core's m-half is always columns 0:2048; the
output columns (l, also rotated) are un-rotated on the host.

On-core layout: everything is computed transposed:
    qT  [Cr, L]  = Wq^T @ xT + bq
    kpT [Cr, M]  = Wk^T @ xTm + (pos + bk)         (pos+bk folded on host)
    vb  [M, C]   = xTm^T @ Wv + bv (rank-1 ones matmul for the bias)
    ST  [M, L]   = kpT^T @ qT      -> exp (ACT, fused colsum accumulation)
    E   [M, L]   fp16 exp(S-11.5), resident in SBUF (16MB)
    vbw [M, C]   = vb * (1/colsum) per row, fp16
    outT[C, L]   = vbw^T @ E       (PSUM accumulation over m-blocks)
"""

import sys

for _p in ("/opt/trn_rl_repo", "/root/.axon_site/_ro/trn_rl_repo"):
    if _p not in sys.path:
        sys.path.append(_p)

import numpy as np

B, L, C, Cr = 4, 4096, 256, 32
MH = L // 2  # per-core key-half size (2048)
NCORES = 8

_CACHE = {}


def build_nc(L=L, C=C, Cr=Cr, M=MH):
    import concourse.bass as bass
    import concourse.tile as tile
    from concourse import mybir
    from concourse.tile_rust import add_dep_helper

    FP32 = mybir.dt.float32
    FP16 = mybir.dt.float16
    Exp = mybir.ActivationFunctionType.Exp
    # E is stored as fp16 exp(S - OFF).  The offset cancels exactly in
    # out = E' @ (v / colsum(E')) and keeps exp(S) inside fp16 range:
    # real-data S in [-19, 19.44], colmax in [3.6, 19.44] -> E' <= e^7.9,
    # vbw' <= |v| * e^{OFF - colmax_min} ~ 4e3, both with >= 16x margin.
    EXP_OFF = 11.5

    assert C == 256 and Cr == 32
    assert L % 1024 == 0 and M % 512 == 0
    NMB = M // 128          # m-blocks per core
    NLG = L // 512          # l-groups for phase 2
    SG = 1024               # phase-1 ACT exp chunk width
    NSG = L // SG           # stats groups per m-block

    # xw (fp16) columns: wq0 0:32 | wq1 32:64 | wk0 64:96 | wk1 96:128 |
    # wv0 128:384 | wv1 384:640 | bq(row0) 640:672 | bv(row0) 672:928 |
    # ones(row0) 928:1440
    nc = bass.Bass()
    xt_d = nc.dram_tensor("xt", [128, 2 * L], FP16, kind="ExternalInput")
    xw_d = nc.dram_tensor("xw", [128, 1440], FP16, kind="ExternalInput")
    pos_d = nc.dram_tensor("pos", [Cr, M], FP32, kind="ExternalInput")
    outT_d = nc.dram_tensor("outT", [C, L], FP32, kind="ExternalOutput")

    with tile.TileContext(nc) as tc:
        with (
            tc.tile_pool(name="persist", bufs=1) as persist,
            tc.tile_pool(name="psum", bufs=1, space="PSUM") as psum,
        ):
            qT = persist.tile([Cr, L], FP16)
            kpT = persist.tile([Cr, M], FP16)
            vb = persist.tile([128, NMB, C], FP16)
            vbw = persist.tile([128, NMB, C], FP16)
            stats = persist.tile([128, NMB, NSG], FP32)
            colsum = persist.tile([128, NMB], FP32)
            wrec = persist.tile([128, NMB], FP32)
            expoff = persist.tile([128, 1], FP32)
            nc.vector.memset(expoff[:], -EXP_OFF)
            # tiny fp16 tile for Ldweights "carrier" instructions: a PE op
            # that takes the cross-engine WAR wait of a PSUM slot being
            # re-opened, so the slot-opening Matmult (1-sem-wait ISA
            # budget) only carries its same-engine bank WAW wait.
            wdum = persist.tile([1, 1], FP16)
            nc.vector.memset(wdum[:], 0.0)

            def carrier(dep):
                if dep is None:
                    return None
                c = nc.tensor.ldweights(wdum[:])
                add_dep_helper(c.ins, dep.ins, sync=True,
                               reason="psum slot WAR carrier")
                return c

            def anchor(mm, c):
                if c is not None:
                    add_dep_helper(mm.ins, c.ins, sync=False,
                                   reason="carrier anchor")
                return mm

            with tc.tile_pool(name="epool", bufs=1) as epool:
                E = epool.tile([128, NMB, L], FP16)

                # ---- prolog pool stays open through phase 1 so vb matmul
                # emission can interleave with the ST/exp stream ----
                with tc.tile_pool(name="prolog", bufs=1) as pp:
                    xw = pp.tile([128, 1440], FP16)
                    nc.sync.dma_start(xw[:], xw_d[:])
                    pos = pp.tile([Cr, M], FP32)
                    nc.gpsimd.dma_start(pos[:], pos_d[:])
                    xt = pp.tile([128, 2, L], FP16)
                    for j in range(L // 1024):
                        for half in range(2):
                            c0 = half * L + j * 1024
                            eng = nc.sync if half == 0 else nc.gpsimd
                            eng.dma_start(xt[:, half, j * 1024:(j + 1) * 1024],
                                          xt_d[:, c0:c0 + 1024])
                    dvew = pp.tile([1, 1], FP32)
                    nc.vector.tensor_copy(dvew[:], pos[0:1, 0:1])

                    wq0, wq1 = xw[:, 0:32], xw[:, 32:64]
                    wk0, wk1 = xw[:, 64:96], xw[:, 96:128]
                    wv0, wv1 = xw[:, 128:384], xw[:, 384:640]
                    bq = xw[0:1, 640:672]
                    bv = xw[0:1, 672:928]
                    ones = xw[0:1, 928:1440]

                    hist_po = [None, None, None, None]
                    hist_st = [None, None]
                    kidx = [0]

                    def po_tile(name):
                        cr_ = carrier(hist_po[kidx[0] % 4])
                        t = psum.tile([128, 512], FP32, tag="po", bufs=4,
                                      name=name)
                        return t, cr_

                    def po_done(reader):
                        hist_po[kidx[0] % 4] = reader
                        kidx[0] += 1

                    # qT / kpT groups are emitted lazily inside the
                    # phase-1 loop so the exp stream starts as soon as the
                    # first chunks are ready (PE executes in queue order).
                    qdone = set()
                    kdone = set()

                    def need_q(j):
                        if j in qdone:
                            return
                        qdone.add(j)
                        sl = slice(j * 512, (j + 1) * 512)
                        psq_t, cr_ = po_tile(f"psq_{j}")
                        psq = psq_t[0:Cr, 0:512]
                        anchor(nc.tensor.matmul(psq[:], wq0, xt[:, 0, sl],
                                                start=True, stop=False), cr_)
                        nc.tensor.matmul(psq[:], wq1, xt[:, 1, sl],
                                         start=False, stop=False)
                        nc.tensor.matmul(psq[:], bq, ones[0:1, 0:512],
                                         start=False, stop=True)
                        po_done(nc.vector.tensor_copy(qT[:, sl], psq[:]))

                    def need_k(j):
                        if j in kdone:
                            return
                        kdone.add(j)
                        sl = slice(j * 512, (j + 1) * 512)
                        psk_t, cr_ = po_tile(f"psk_{j}")
                        psk = psk_t[0:Cr, 0:512]
                        anchor(nc.tensor.matmul(psk[:], wk0, xt[:, 0, sl],
                                                start=True, stop=False), cr_)
                        nc.tensor.matmul(psk[:], wk1, xt[:, 1, sl],
                                         start=False, stop=True)
                        po_done(nc.vector.tensor_add(
                            kpT[:, sl], psk[:], pos[:, sl]))

                    # ---- phase 1 (vb groups emitted after 4 m-blocks so
                    # the exp stream starts immediately; vb[mb] is only
                    # needed after mb's colsum) ----
                    last_vb = {}

                    def stats_tail(mb):
                        nc.vector.reduce_sum(colsum[:, mb:mb + 1],
                                             stats[:, mb, :],
                                             axis=mybir.AxisListType.X)
                        nc.vector.reciprocal(wrec[:, mb:mb + 1],
                                             colsum[:, mb:mb + 1])
                        nc.vector.tensor_scalar_mul(
                            vbw[:, mb, :], vb[:, mb, :], wrec[:, mb:mb + 1])

                    VB_AT = min(4, NMB - 1)
                    for mb in range(NMB):
                        if mb % 4 == 0:
                            need_k(mb // 4)
                        if mb == VB_AT:
                            for vmb in range(NMB):
                                msl = slice(vmb * 128, (vmb + 1) * 128)
                                psv_t, cr_ = po_tile(f"psv_{vmb}")
                                psv = psv_t[:, 0:C]
                                anchor(nc.tensor.matmul(
                                    psv[:], xt[:, 0, msl], wv0,
                                    start=True, stop=False), cr_)
                                nc.tensor.matmul(psv[:], xt[:, 1, msl], wv1,
                                                 start=False, stop=False)
                                nc.tensor.matmul(psv[:], ones[0:1, 0:128],
                                                 bv, start=False, stop=True)
                                lvb = nc.vector.tensor_copy(
                                    vb[:, vmb, :], psv[:])
                                po_done(lvb)
                                last_vb[vmb] = lvb
                            # emit the deferred stats tails now that vb exists
                            for pmb in range(VB_AT):
                                stats_tail(pmb)
                        kp_sl = kpT[:, mb * 128:(mb + 1) * 128]
                        for g in range(NSG):
                            if mb == 0:
                                need_q(2 * g)
                                need_q(2 * g + 1)
                            idx = mb * NSG + g
                            cr_ = carrier(hist_st[idx % 2])
                            ps = psum.tile([128, SG], FP32, tag="st", bufs=2)
                            last = None
                            for j in range(SG // 512):
                                lsl = slice(g * SG + j * 512,
                                            g * SG + (j + 1) * 512)
                                last = nc.tensor.matmul(
                                    ps[:, j * 512:(j + 1) * 512],
                                    kp_sl, qT[:, lsl], start=True, stop=True)
                                if j == 0:
                                    anchor(last, cr_)
                            last_exp = nc.scalar.activation(
                                E[:, mb, g * SG:(g + 1) * SG], ps[:], Exp,
                                bias=expoff[:],
                                accum_out=stats[:, mb, g:g + 1])
                            hist_st[idx % 2] = last_exp
                        if mb >= VB_AT:
                            stats_tail(mb)

                # ---- phase 2 (prolog closed; stage pools reuse its zone).
                # Split m-accumulation: partA = mb 0..HM-1 closes mid
                # phase-1; partB quarters close at 3/4 and at the end. ----
                HM = NMB // 2
                with (
                    tc.tile_pool(name="stagea", bufs=16) as stagea,
                    tc.tile_pool(name="stage", bufs=4) as stage,
                ):
                    soas = []
                    for lg in range(NLG):
                        lsl = slice(lg * 512, (lg + 1) * 512)
                        for ch in range(C // 128):
                            poa, cr_ = po_tile(f"poa_{lg}_{ch}")
                            last = None
                            for mb in range(HM):
                                last = nc.tensor.matmul(
                                    poa[:],
                                    vbw[:, mb, ch * 128:(ch + 1) * 128],
                                    E[:, mb, lsl],
                                    start=(mb == 0), stop=(mb == HM - 1))
                                if mb == 0:
                                    anchor(last, cr_)
                            soa = stagea.tile([128, 512], FP16, tag="soa",
                                              name=f"soa_{lg}_{ch}")
                            po_done(nc.vector.tensor_copy(soa[:], poa[:]))
                            soas.append(soa)

                    QR = NMB // 4
                    splits = [(HM, HM + QR), (HM + QR, NMB)]
                    splits = [(a, b) for a, b in splits if b > a]
                    for q, (m0, m1) in enumerate(splits):
                        for lg in range(NLG):
                            lsl = slice(lg * 512, (lg + 1) * 512)
                            for ch in range(C // 128):
                                pob, cr_ = po_tile(f"pob_{q}_{lg}_{ch}")
                                last = None
                                for mb in range(m0, m1):
                                    last = nc.tensor.matmul(
                                        pob[:],
                                        vbw[:, mb, ch * 128:(ch + 1) * 128],
                                        E[:, mb, lsl],
                                        start=(mb == m0), stop=(mb == m1 - 1))
                                    if mb == m0:
                                        anchor(last, cr_)
                                soa = soas[lg * 2 + ch]
                                if q < len(splits) - 1:
                                    po_done(nc.vector.tensor_add(
                                        soa[:], pob[:], soa[:]))
                                else:
                                    so = stage.tile([128, 512], FP32,
                                                    tag="so")
                                    po_done(nc.vector.tensor_add(
                                        so[:], pob[:], soa[:]))
                                    nc.sync.dma_start(
                                        outT_d[ch * 128:(ch + 1) * 128, lsl],
                                        so[:])

    return nc


def _fixup_waits(nc):
    """Walrus codegen on this toolchain allows only ~1 semaphore wait per
    TPB instruction (2 for DMACopy).  Hoist excess waits into standalone
    single-wait EventSemaphore instructions inserted just before the
    over-budget instruction on the same engine (same-stream ordering makes
    this semantics-preserving)."""
    from concourse import mybir

    budget_by_type = {}
    n = 0
    for fn in nc.m.functions:
        for blk in fn.blocks:
            insts = blk.instructions
            i = 0
            while i < len(insts):
                inst = insts[i]
                si = getattr(inst, "sync_info", None)
                if si is None:
                    i += 1
                    continue
                waits = list(si.on_wait)
                budget = budget_by_type.get(type(inst).__name__, 1)
                if len(waits) <= budget:
                    i += 1
                    continue
                extra, keep = waits[:-budget], waits[-budget:]
                for w in extra:
                    es = mybir.InstEventSemaphore(
                        name=f"es_waitfix_{n}", ins=[], outs=[])
                    n += 1
                    es.engine = inst.engine
                    es.sync_info = mybir.SyncInfo(on_wait=[w], on_update=[])
                    insts.insert(i, es)
                    i += 1
                inst.sync_info = mybir.SyncInfo(
                    on_wait=keep, on_update=list(si.on_update))
                i += 1


def _build_and_fix(**kw):
    nc = build_nc(**kw)
    _fixup_waits(nc)
    return nc


def _get_nc(key, **kw):
    if key not in _CACHE:
        _CACHE[key] = _build_and_fix(**kw)
    return _CACHE[key]


def _prep_core_inputs(x, rel_h, rel_w, Wq, bq, Wk, bk, Wv, bv):
    """Build the 8 per-core input maps (host-side sharding / layout prep)."""
    x = np.asarray(x, dtype=np.float32)
    pos = (np.asarray(rel_h, np.float32) + np.asarray(rel_w, np.float32))
    pos = pos.reshape(Cr, L) + np.asarray(bk, np.float32).reshape(Cr, 1)
    xw = np.zeros((128, 1440), np.float16)
    xw[:, 0:32] = np.asarray(Wq, np.float32)[0:128]
    xw[:, 32:64] = np.asarray(Wq, np.float32)[128:256]
    xw[:, 64:96] = np.asarray(Wk, np.float32)[0:128]
    xw[:, 96:128] = np.asarray(Wk, np.float32)[128:256]
    xw[:, 128:384] = np.asarray(Wv, np.float32)[0:128]
    xw[:, 384:640] = np.asarray(Wv, np.float32)[128:256]
    xw[0, 640:672] = np.asarray(bq, np.float32).ravel()
    xw[0, 672:928] = np.asarray(bv, np.float32).ravel()
    xw[0, 928:1440] = 1.0

    in_maps = []
    for i in range(NCORES):
        b, h = divmod(i, 2)
        xT = x[b].T.astype(np.float16)  # [C, L]
        if h == 1:
            xT = np.concatenate([xT[:, MH:], xT[:, :MH]], axis=1)
        xtc = np.ascontiguousarray(
            np.concatenate([xT[0:128], xT[128:256]], axis=1))
        posh = np.ascontiguousarray(pos[:, h * MH:(h + 1) * MH])
        in_maps.append({"xt": xtc, "xw": xw, "pos": posh})
    return in_maps


def _combine(results):
    """results: list of 8 out_maps -> full [B, L, C] output."""
    out = np.empty((B, L, C), dtype=np.float32)
    for b in range(B):
        o0 = results[2 * b]["outT"]          # [C, L], true l order
        o1 = results[2 * b + 1]["outT"]      # [C, L], l rotated by MH
        o1 = np.concatenate([o1[:, MH:], o1[:, :MH]], axis=1)
        out[b] = (o0 + o1).T
    return out


def kernel(**inputs):
    from concourse.bass_utils import run_bass_kernel_spmd

    nc = _get_nc("full")
    in_maps = _prep_core_inputs(**inputs)
    res = run_bass_kernel_spmd(nc, in_maps, core_ids=list(range(NCORES)))
    return _combine(res.results)


if __name__ == "__main__":
    rng = np.random.default_rng(0)
    ins = {
        "x": rng.standard_normal((B, L, C), dtype=np.float32),
        "rel_h": rng.standard_normal((1, Cr, 64, 1), dtype=np.float32),
        "rel_w": rng.standard_normal((1, Cr, 1, 64), dtype=np.float32),
        "Wq": rng.standard_normal((C, Cr), dtype=np.float32) * 0.02,
        "bq": np.zeros(Cr, np.float32),
        "Wk": rng.standard_normal((C, Cr), dtype=np.float32) * 0.02,
        "bk": np.zeros(Cr, np.float32),
        "Wv": rng.standard_normal((C, C), dtype=np.float32) * 0.02,
        "bv": np.zeros(C, np.float32),
    }
    out = kernel(**ins)
    print(out.shape, out.dtype)


# revision 45
# speedup vs baseline: 1.5284x; 1.0042x over previous
"""Trainium2 Bass kernel for nn_Att_61881888801149 (sparse_attention).

Math (per batch b):
    q = x @ Wq + bq                  [L, Cr]
    k = x @ Wk + bk                  [L, Cr]
    v = x @ Wv + bv                  [L, C]
    pos = (rel_h + rel_w).reshape(Cr, L)
    S = q @ (k^T + pos)              [L, L]   (queries l, keys m)
    attn = softmax(S, axis=0)        (normalized over the QUERY axis l)
    out = attn @ v                   [L, C]

Because the softmax axis (l) is orthogonal to the bmm contraction axis (m):
    out[l, c] = sum_m  E[l, m] * v[m, c] / colsum[m]
with E = exp(S) (no max subtraction needed - scores are small), and
colsum[m] = sum_l E[l, m].

Sharding: 8 cores = 4 batches x 2 key-halves (m in [0,2048) or [2048,4096)).
Host sums the two partial outputs per batch.  SPMD trick: the host rotates
xT's columns per-core so each core's m-half is always columns 0:2048; the
output columns (l, also rotated) are un-rotated on the host.

On-core layout: everything is computed transposed:
    qT  [Cr, L]  = Wq^T @ xT + bq
    kpT [Cr, M]  = Wk^T @ xTm + (pos + bk)         (pos+bk folded on host)
    vb  [M, C]   = xTm^T @ Wv + bv (rank-1 ones matmul for the bias)
    ST  [M, L]   = kpT^T @ qT      -> exp (ACT, fused colsum accumulation)
    E   [M, L]   fp16 exp(S-11.5), resident in SBUF (16MB)
    vbw [M, C]   = vb * (1/colsum) per row, fp16
    outT[C, L]   = vbw^T @ E       (PSUM accumulation over m-blocks)
"""

import sys

for _p in ("/opt/trn_rl_repo", "/root/.axon_site/_ro/trn_rl_repo"):
    if _p not in sys.path:
        sys.path.append(_p)

import numpy as np

B, L, C, Cr = 4, 4096, 256, 32
MH = L // 2  # per-core key-half size (2048)
NCORES = 8

_CACHE = {}


def build_nc(L=L, C=C, Cr=Cr, M=MH):
    import concourse.bass as bass
    import concourse.tile as tile
    from concourse import mybir
    from concourse.tile_rust import add_dep_helper

    FP32 = mybir.dt.float32
    FP16 = mybir.dt.float16
    Exp = mybir.ActivationFunctionType.Exp
    # E is stored as fp16 exp(S - OFF).  The offset cancels exactly in
    # out = E' @ (v / colsum(E')) and keeps exp(S) inside fp16 range:
    # real-data S in [-19, 19.44], colmax in [3.6, 19.44] -> E' <= e^7.9,
    # vbw' <= |v| * e^{OFF - colmax_min} ~ 4e3, both with >= 16x margin.
    EXP_OFF = 11.5

    assert C == 256 and Cr == 32
    assert L % 1024 == 0 and M % 512 == 0
    NMB = M // 128          # m-blocks per core
    NLG = L // 512          # l-groups for phase 2
    SG = 1024               # phase-1 ACT exp chunk width
    NSG = L // SG           # stats groups per m-block

    # xw (fp16) columns: wq0 0:32 | wq1 32:64 | wk0 64:96 | wk1 96:128 |
    # wv0 128:384 | wv1 384:640 | bq(row0) 640:672 | bv(row0) 672:928 |
    # ones(row0) 928:1440
    nc = bass.Bass()
    xt_d = nc.dram_tensor("xt", [128, 2 * L], FP16, kind="ExternalInput")
    xw_d = nc.dram_tensor("xw", [128, 1440], FP16, kind="ExternalInput")
    pos_d = nc.dram_tensor("pos", [Cr, M], FP32, kind="ExternalInput")
    outT_d = nc.dram_tensor("outT", [C, L], FP32, kind="ExternalOutput")

    with tile.TileContext(nc) as tc:
        with (
            tc.tile_pool(name="persist", bufs=1) as persist,
            tc.tile_pool(name="psum", bufs=1, space="PSUM") as psum,
        ):
            qT = persist.tile([Cr, L], FP16)
            kpT = persist.tile([Cr, M], FP16)
            vb = persist.tile([128, NMB, C], FP16)
            vbw = persist.tile([128, NMB, C], FP16)
            stats = persist.tile([128, NMB, NSG], FP32)
            colsum = persist.tile([128, NMB], FP32)
            wrec = persist.tile([128, NMB], FP32)
            expoff = persist.tile([128, 1], FP32)
            nc.vector.memset(expoff[:], -EXP_OFF)
            # tiny fp16 tile for Ldweights "carrier" instructions: a PE op
            # that takes the cross-engine WAR wait of a PSUM slot being
            # re-opened, so the slot-opening Matmult (1-sem-wait ISA
            # budget) only carries its same-engine bank WAW wait.
            wdum = persist.tile([1, 1], FP16)
            nc.vector.memset(wdum[:], 0.0)
            # warm the ACT exp table (~2.6us load) before the exp stream
            exw = persist.tile([1, 1], FP32)
            nc.scalar.activation(exw[:], expoff[0:1, 0:1], Exp)

            def carrier(dep):
                if dep is None:
                    return None
                c = nc.tensor.ldweights(wdum[:])
                add_dep_helper(c.ins, dep.ins, sync=True,
                               reason="psum slot WAR carrier")
                return c

            def anchor(mm, c):
                if c is not None:
                    add_dep_helper(mm.ins, c.ins, sync=False,
                                   reason="carrier anchor")
                return mm

            with tc.tile_pool(name="epool", bufs=1) as epool:
                E = epool.tile([128, NMB, L], FP16)

                # ---- prolog pool stays open through phase 1 so vb matmul
                # emission can interleave with the ST/exp stream ----
                with tc.tile_pool(name="prolog", bufs=1) as pp:
                    xw = pp.tile([128, 1440], FP16)
                    nc.sync.dma_start(xw[:], xw_d[:])
                    pos = pp.tile([Cr, M], FP32)
                    nc.gpsimd.dma_start(pos[:], pos_d[:])
                    xt = pp.tile([128, 2, L], FP16)
                    for j in range(L // 1024):
                        for half in range(2):
                            c0 = half * L + j * 1024
                            eng = nc.sync if half == 0 else nc.gpsimd
                            eng.dma_start(xt[:, half, j * 1024:(j + 1) * 1024],
                                          xt_d[:, c0:c0 + 1024])
                    dvew = pp.tile([1, 1], FP32)
                    nc.vector.tensor_copy(dvew[:], pos[0:1, 0:1])

                    wq0, wq1 = xw[:, 0:32], xw[:, 32:64]
                    wk0, wk1 = xw[:, 64:96], xw[:, 96:128]
                    wv0, wv1 = xw[:, 128:384], xw[:, 384:640]
                    bq = xw[0:1, 640:672]
                    bv = xw[0:1, 672:928]
                    ones = xw[0:1, 928:1440]

                    hist_po = [None, None, None, None]
                    hist_st = [None, None]
                    kidx = [0]

                    def po_tile(name):
                        cr_ = carrier(hist_po[kidx[0] % 4])
                        t = psum.tile([128, 512], FP32, tag="po", bufs=4,
                                      name=name)
                        return t, cr_

                    def po_done(reader):
                        hist_po[kidx[0] % 4] = reader
                        kidx[0] += 1

                    # qT / kpT groups are emitted lazily inside the
                    # phase-1 loop so the exp stream starts as soon as the
                    # first chunks are ready (PE executes in queue order).
                    qdone = set()
                    kdone = set()

                    def need_q(j):
                        if j in qdone:
                            return
                        qdone.add(j)
                        sl = slice(j * 512, (j + 1) * 512)
                        psq_t, cr_ = po_tile(f"psq_{j}")
                        psq = psq_t[0:Cr, 0:512]
                        anchor(nc.tensor.matmul(psq[:], wq0, xt[:, 0, sl],
                                                start=True, stop=False), cr_)
                        nc.tensor.matmul(psq[:], wq1, xt[:, 1, sl],
                                         start=False, stop=False)
                        nc.tensor.matmul(psq[:], bq, ones[0:1, 0:512],
                                         start=False, stop=True)
                        po_done(nc.vector.tensor_copy(qT[:, sl], psq[:]))

                    def need_k(j):
                        if j in kdone:
                            return
                        kdone.add(j)
                        sl = slice(j * 512, (j + 1) * 512)
                        psk_t, cr_ = po_tile(f"psk_{j}")
                        psk = psk_t[0:Cr, 0:512]
                        anchor(nc.tensor.matmul(psk[:], wk0, xt[:, 0, sl],
                                                start=True, stop=False), cr_)
                        nc.tensor.matmul(psk[:], wk1, xt[:, 1, sl],
                                         start=False, stop=True)
                        po_done(nc.vector.tensor_add(
                            kpT[:, sl], psk[:], pos[:, sl]))

                    # ---- phase 1 (vb groups emitted after 4 m-blocks so
                    # the exp stream starts immediately; vb[mb] is only
                    # needed after mb's colsum) ----
                    last_vb = {}

                    def stats_tail(mb):
                        nc.vector.reduce_sum(colsum[:, mb:mb + 1],
                                             stats[:, mb, :],
                                             axis=mybir.AxisListType.X)
                        nc.vector.reciprocal(wrec[:, mb:mb + 1],
                                             colsum[:, mb:mb + 1])
                        nc.vector.tensor_scalar_mul(
                            vbw[:, mb, :], vb[:, mb, :], wrec[:, mb:mb + 1])

                    VB_AT = min(4, NMB - 1)
                    for mb in range(NMB):
                        if mb % 4 == 0:
                            need_k(mb // 4)
                        if mb == VB_AT:
                            for vmb in range(NMB):
                                msl = slice(vmb * 128, (vmb + 1) * 128)
                                psv_t, cr_ = po_tile(f"psv_{vmb}")
                                psv = psv_t[:, 0:C]
                                anchor(nc.tensor.matmul(
                                    psv[:], xt[:, 0, msl], wv0,
                                    start=True, stop=False), cr_)
                                nc.tensor.matmul(psv[:], xt[:, 1, msl], wv1,
                                                 start=False, stop=False)
                                nc.tensor.matmul(psv[:], ones[0:1, 0:128],
                                                 bv, start=False, stop=True)
                                lvb = nc.vector.tensor_copy(
                                    vb[:, vmb, :], psv[:])
                                po_done(lvb)
                                last_vb[vmb] = lvb
                            # emit the deferred stats tails now that vb exists
                            for pmb in range(VB_AT):
                                stats_tail(pmb)
                        kp_sl = kpT[:, mb * 128:(mb + 1) * 128]
                        for g in range(NSG):
                            if mb == 0:
                                need_q(2 * g)
                                need_q(2 * g + 1)
                            idx = mb * NSG + g
                            cr_ = carrier(hist_st[idx % 2])
                            ps = psum.tile([128, SG], FP32, tag="st", bufs=2)
                            last = None
                            for j in range(SG // 512):
                                lsl = slice(g * SG + j * 512,
                                            g * SG + (j + 1) * 512)
                                last = nc.tensor.matmul(
                                    ps[:, j * 512:(j + 1) * 512],
                                    kp_sl, qT[:, lsl], start=True, stop=True)
                                if j == 0:
                                    anchor(last, cr_)
                            last_exp = nc.scalar.activation(
                                E[:, mb, g * SG:(g + 1) * SG], ps[:], Exp,
                                bias=expoff[:],
                                accum_out=stats[:, mb, g:g + 1])
                            hist_st[idx % 2] = last_exp
                        if mb >= VB_AT:
                            stats_tail(mb)

                # ---- phase 2 (prolog closed; stage pools reuse its zone).
                # Split m-accumulation: partA = mb 0..HM-1 closes mid
                # phase-1; partB quarters close at 3/4 and at the end. ----
                HM = NMB // 2
                with (
                    tc.tile_pool(name="stagea", bufs=16) as stagea,
                    tc.tile_pool(name="stage", bufs=4) as stage,
                ):
                    soas = []
                    for lg in range(NLG):
                        lsl = slice(lg * 512, (lg + 1) * 512)
                        for ch in range(C // 128):
                            poa, cr_ = po_tile(f"poa_{lg}_{ch}")
                            last = None
                            for mb in range(HM):
                                last = nc.tensor.matmul(
                                    poa[:],
                                    vbw[:, mb, ch * 128:(ch + 1) * 128],
                                    E[:, mb, lsl],
                                    start=(mb == 0), stop=(mb == HM - 1))
                                if mb == 0:
                                    anchor(last, cr_)
                            soa = stagea.tile([128, 512], FP16, tag="soa",
                                              name=f"soa_{lg}_{ch}")
                            po_done(nc.vector.tensor_copy(soa[:], poa[:]))
                            soas.append(soa)

                    QR = NMB // 4
                    splits = [(HM, HM + QR), (HM + QR, NMB)]
                    splits = [(a, b) for a, b in splits if b > a]
                    for q, (m0, m1) in enumerate(splits):
                        for lg in range(NLG):
                            lsl = slice(lg * 512, (lg + 1) * 512)
                            for ch in range(C // 128):
                                pob, cr_ = po_tile(f"pob_{q}_{lg}_{ch}")
                                last = None
                                for mb in range(m0, m1):
                                    last = nc.tensor.matmul(
                                        pob[:],
                                        vbw[:, mb, ch * 128:(ch + 1) * 128],
                                        E[:, mb, lsl],
                                        start=(mb == m0), stop=(mb == m1 - 1))
                                    if mb == m0:
                                        anchor(last, cr_)
                                soa = soas[lg * 2 + ch]
                                if q < len(splits) - 1:
                                    po_done(nc.vector.tensor_add(
                                        soa[:], pob[:], soa[:]))
                                else:
                                    so = stage.tile([128, 512], FP32,
                                                    tag="so")
                                    po_done(nc.vector.tensor_add(
                                        so[:], pob[:], soa[:]))
                                    nc.sync.dma_start(
                                        outT_d[ch * 128:(ch + 1) * 128, lsl],
                                        so[:])

    return nc


def _fixup_waits(nc):
    """Walrus codegen on this toolchain allows only ~1 semaphore wait per
    TPB instruction (2 for DMACopy).  Hoist excess waits into standalone
    single-wait EventSemaphore instructions inserted just before the
    over-budget instruction on the same engine (same-stream ordering makes
    this semantics-preserving)."""
    from concourse import mybir

    budget_by_type = {}
    n = 0
    for fn in nc.m.functions:
        for blk in fn.blocks:
            insts = blk.instructions
            i = 0
            while i < len(insts):
                inst = insts[i]
                si = getattr(inst, "sync_info", None)
                if si is None:
                    i += 1
                    continue
                waits = list(si.on_wait)
                budget = budget_by_type.get(type(inst).__name__, 1)
                if len(waits) <= budget:
                    i += 1
                    continue
                extra, keep = waits[:-budget], waits[-budget:]
                for w in extra:
                    es = mybir.InstEventSemaphore(
                        name=f"es_waitfix_{n}", ins=[], outs=[])
                    n += 1
                    es.engine = inst.engine
                    es.sync_info = mybir.SyncInfo(on_wait=[w], on_update=[])
                    insts.insert(i, es)
                    i += 1
                inst.sync_info = mybir.SyncInfo(
                    on_wait=keep, on_update=list(si.on_update))
                i += 1


def _build_and_fix(**kw):
    nc = build_nc(**kw)
    _fixup_waits(nc)
    return nc


def _get_nc(key, **kw):
    if key not in _CACHE:
        _CACHE[key] = _build_and_fix(**kw)
    return _CACHE[key]


def _prep_core_inputs(x, rel_h, rel_w, Wq, bq, Wk, bk, Wv, bv):
    """Build the 8 per-core input maps (host-side sharding / layout prep)."""
    x = np.asarray(x, dtype=np.float32)
    pos = (np.asarray(rel_h, np.float32) + np.asarray(rel_w, np.float32))
    pos = pos.reshape(Cr, L) + np.asarray(bk, np.float32).reshape(Cr, 1)
    xw = np.zeros((128, 1440), np.float16)
    xw[:, 0:32] = np.asarray(Wq, np.float32)[0:128]
    xw[:, 32:64] = np.asarray(Wq, np.float32)[128:256]
    xw[:, 64:96] = np.asarray(Wk, np.float32)[0:128]
    xw[:, 96:128] = np.asarray(Wk, np.float32)[128:256]
    xw[:, 128:384] = np.asarray(Wv, np.float32)[0:128]
    xw[:, 384:640] = np.asarray(Wv, np.float32)[128:256]
    xw[0, 640:672] = np.asarray(bq, np.float32).ravel()
    xw[0, 672:928] = np.asarray(bv, np.float32).ravel()
    xw[0, 928:1440] = 1.0

    in_maps = []
    for i in range(NCORES):
        b, h = divmod(i, 2)
        xT = x[b].T.astype(np.float16)  # [C, L]
        if h == 1:
            xT = np.concatenate([xT[:, MH:], xT[:, :MH]], axis=1)
        xtc = np.ascontiguousarray(
            np.concatenate([xT[0:128], xT[128:256]], axis=1))
        posh = np.ascontiguousarray(pos[:, h * MH:(h + 1) * MH])
        in_maps.append({"xt": xtc, "xw": xw, "pos": posh})
    return in_maps


def _combine(results):
    """results: list of 8 out_maps -> full [B, L, C] output."""
    out = np.empty((B, L, C), dtype=np.float32)
    for b in range(B):
        o0 = results[2 * b]["outT"]          # [C, L], true l order
        o1 = results[2 * b + 1]["outT"]      # [C, L], l rotated by MH
        o1 = np.concatenate([o1[:, MH:], o1[:, :MH]], axis=1)
        out[b] = (o0 + o1).T
    return out


def kernel(**inputs):
    from concourse.bass_utils import run_bass_kernel_spmd

    nc = _get_nc("full")
    in_maps = _prep_core_inputs(**inputs)
    res = run_bass_kernel_spmd(nc, in_maps, core_ids=list(range(NCORES)))
    return _combine(res.results)


if __name__ == "__main__":
    rng = np.random.default_rng(0)
    ins = {
        "x": rng.standard_normal((B, L, C), dtype=np.float32),
        "rel_h": rng.standard_normal((1, Cr, 64, 1), dtype=np.float32),
        "rel_w": rng.standard_normal((1, Cr, 1, 64), dtype=np.float32),
        "Wq": rng.standard_normal((C, Cr), dtype=np.float32) * 0.02,
        "bq": np.zeros(Cr, np.float32),
        "Wk": rng.standard_normal((C, Cr), dtype=np.float32) * 0.02,
        "bk": np.zeros(Cr, np.float32),
        "Wv": rng.standard_normal((C, C), dtype=np.float32) * 0.02,
        "bv": np.zeros(C, np.float32),
    }
    out = kernel(**ins)
    print(out.shape, out.dtype)
